# revision 24
# baseline (speedup 1.0000x reference)
"""Bass/Trainium2 kernel for nn_BboxIoULoss (topk_masking).

Computes, for S=64 samples / M=1024 targets / P=8256 triu proposals:
    loss = sum((1 - diou) * mask) / sum(mask)
where mask = topk-scatter(3) OR (iou1ds > 0.5), iou1ds = triu-gather of
iou2ds, and diou is the 1-D DIoU between each target and the per-sample
proposal moments.

Strategy (8 NeuronCores, data-parallel over M):
  - core k handles targets m in [128k, 128(k+1)); partition p = local
    target; each partition's sample is p // 16.
  - host prep (sharding/layout only): triu-compact iou2ds to p-order,
    cast to bf16; replicate per-sample moments s1/e1/c1=s1+e1 (bf16)
    across each sample's 16 target partitions; chunk-major slabs so
    each chunk is two contiguous DMAs (iou | moments).
  - device math per chunk (bf16; union == enclose whenever the boxes
    overlap, so iou = relu(w)/enc), tiles reused in place:
        mask = is_gt(iou, 0.5)                 DVE ts (4x)
        rowA += sum(mask)                      ACT copy (+accum, in place)
        u    = max(s1, s2)                     DVE ts
        v    = min(e1, e2)                     DVE ts
        v    = v - u          (w)              DVE tt (2x)
        mn   = min(s1, s2)                     DVE ts
        mx   = max(e1, e2)                     DVE ts
        mx   = mx - mn        (enc)            DVE tt
        lg   = Ln(mx)                          ACT
        renc = Exp(-lg)       (1/enc)          ACT (~2 ULP splines)
        mask = mask * renc    (rm)             DVE tt
        v    = max(v, 0)      (rw)             DVE ts
        cd   = c1 - c2                         DVE ts
        cd   = cd * rm        (qm)             DVE tt
    and the two big sums go to the otherwise-idle TensorEngine as
    block-diagonal PSUM accumulations over 128-column blocks:
        PB1 += rw_blk^T @ rm_blk      (diag holds sum(rw*rm) partials)
        PB2 += qm_blk^T @ qm_blk      (diag holds sum(qm^2) partials)
    B1 = trace(PB1) = sum(mask*iou), B2q = trace(PB2) = 4*sum(mask*pen),
        answer = (A - B1 + B2q/4) / A   (host sums in float64).
  - the top-3 scatter is subsumed by the threshold whenever every row
    has >= 3 entries above 0.5 (then the top-3 values are all > 0.5).
    The device returns per-row counts; if any row has < 3, or
    num_targets is not uniform, a numpy fallback reproduces the
    reference exactly.
"""

import os
import ml_dtypes
import numpy as np

import concourse.bass as bass
import concourse.tile as tile
import concourse.mybir as mybir
from concourse import bacc, bass_utils

F32 = mybir.dt.float32
BF16 = mybir.dt.bfloat16
AF = mybir.ActivationFunctionType
OP = mybir.AluOpType

S = 64
T = 16
N = 128
M = S * T                  # 1024
P = N * (N + 1) // 2       # 8256
TOPK = 3
IOU_THRESHOLD = 0.5
NCORES = 8
ML = M // NCORES           # 128 targets / core (= partitions)
W = S // NCORES            # 8 samples / core
NCH = int(os.environ.get("BBK_NCH", "4"))
CH = P // NCH              # free-dim chunk
BLK = 128                  # matmul block width


def _patch_act_tables():
    """Force one activation table-set (has ln/exp/relu/square/copy) so the
    scheduler emits a single ACT_TABLE_LOAD instead of thrashing sets."""
    import concourse.bacc as _bacc
    orig = _bacc.get_activation_tables

    def only_lnexp(arch):
        tabs = orig(arch)
        name = "natural_log_exp_and_others"
        if name not in tabs:
            return tabs
        return {k: (v if k == name else set()) for k, v in tabs.items()}

    _bacc.get_activation_tables = only_lnexp


def _build_program():
    if not os.environ.get("BBK_NOPATCH"):
        _patch_act_tables()
    nc = bacc.Bacc(
        "TRN2", target_bir_lowering=False, debug=False, enable_asserts=False
    )
    # chunk-major slabs: rows [c*ML, (c+1)*ML) = chunk c
    iou_d = nc.dram_tensor("iou", [NCH * ML, CH], BF16, kind="ExternalInput")
    mo_d = nc.dram_tensor("mo", [NCH * ML, 3 * CH], BF16, kind="ExternalInput")
    tgt_d = nc.dram_tensor("tgt", [ML, 3], F32, kind="ExternalInput")
    # acc: cols [0:NCH] = per-chunk rowA
    acc_d = nc.dram_tensor("acc", [ML, NCH], F32, kind="ExternalOutput")
    # [PB1 | PB2] block-diagonal gram accumulators (host takes traces)
    mm_d = nc.dram_tensor("mm", [ML, 2 * BLK], F32, kind="ExternalOutput")

    linearize = bool(int(os.environ.get("BBK_LINEARIZE", "0")))
    with tile.TileContext(nc, linearize=linearize) as tc:
        with (
            tc.tile_pool(name="const", bufs=1) as cp,
            tc.tile_pool(name="inp", bufs=int(os.environ.get("BBK_IBUFS", "3"))) as ip,
            tc.tile_pool(name="work", bufs=int(os.environ.get("BBK_WBUFS", "3"))) as wp,
            tc.psum_pool(name="ps", bufs=1) as pp,
        ):
            tgt = cp.tile([ML, 3], F32)
            s2 = tgt[:, 0:1]
            e2 = tgt[:, 1:2]
            c2 = tgt[:, 2:3]

            acc = cp.tile([ML, NCH], F32)
            pb1 = pp.tile([ML, BLK], F32)
            pb2 = pp.tile([ML, BLK], F32)

            cd_eng = os.environ.get("BBK_CD_ENGINE", "vector")
            nblk = (CH + BLK - 1) // BLK
            for c in range(NCH):
                iot = ip.tile([ML, CH], BF16, tag="iot")
                nc.sync.dma_start(
                    iot[:], iou_d.ap()[c * ML : (c + 1) * ML, :]
                )
                mot = ip.tile([ML, 3 * CH], BF16, tag="mot")
                # split at the c1 boundary: u/v/mn/mx gate only on s1|e1,
                # so the chain starts before c1 (consumed late by cd) lands
                nc.sync.dma_start(
                    mot[:, 0 : 2 * CH],
                    mo_d.ap()[c * ML : (c + 1) * ML, 0 : 2 * CH],
                )
                nc.sync.dma_start(
                    mot[:, 2 * CH : 3 * CH],
                    mo_d.ap()[c * ML : (c + 1) * ML, 2 * CH : 3 * CH],
                )
                if c == 0:
                    # after the chunk-0 slabs so their transfers start first
                    nc.sync.dma_start(tgt[:], tgt_d.ap())
                s1 = mot[:, 0:CH]
                e1 = mot[:, CH : 2 * CH]
                c1 = mot[:, 2 * CH : 3 * CH]

                # mask -> (in place) rowA accum -> (in place) rm
                mask = wp.tile([ML, CH], BF16, tag="mask")
                nc.vector.tensor_scalar(
                    mask[:], iot[:], IOU_THRESHOLD, None, OP.is_gt
                )
                nc.scalar.activation(
                    mask[:], mask[:], AF.Copy,
                    accum_out=acc[:, c : c + 1],
                )

                u = wp.tile([ML, CH], BF16, tag="u")
                nc.vector.tensor_scalar(u[:], s1, s2, None, OP.max)
                v = wp.tile([ML, CH], BF16, tag="v")
                nc.vector.tensor_scalar(v[:], e1, e2, None, OP.min)
                nc.vector.tensor_tensor(v[:], v[:], u[:], OP.subtract)  # w
                mn = wp.tile([ML, CH], BF16, tag="mn")
                nc.vector.tensor_scalar(mn[:], s1, s2, None, OP.min)
                mx = wp.tile([ML, CH], BF16, tag="mx")
                nc.vector.tensor_scalar(mx[:], e1, e2, None, OP.max)
                nc.vector.tensor_tensor(mx[:], mx[:], mn[:], OP.subtract)  # enc

                lg = wp.tile([ML, CH], F32, tag="lg")
                nc.scalar.activation(lg[:], mx[:], AF.Ln)
                renc = wp.tile([ML, CH], BF16, tag="renc")
                nc.scalar.activation(renc[:], lg[:], AF.Exp, scale=-1.0)

                if os.environ.get("BBK_RM_ENGINE", "vector") == "gpsimd":
                    nc.gpsimd.tensor_tensor(mask[:], mask[:], renc[:], OP.mult)
                else:
                    nc.vector.tensor_tensor(mask[:], mask[:], renc[:], OP.mult)
                rm = mask  # rm overwrote mask in place

                cd = wp.tile([ML, CH], BF16, tag="cd")
                if cd_eng == "gpsimd":
                    nc.gpsimd.tensor_scalar(cd[:], c1, c2, None, OP.subtract)
                else:
                    nc.vector.tensor_scalar(cd[:], c1, c2, None, OP.subtract)
                nc.vector.tensor_tensor(cd[:], cd[:], rm[:], OP.mult)  # qm
                qm = cd

                nc.vector.tensor_scalar(v[:], v[:], 0.0, None, OP.max)  # rw
                for b in range(nblk):
                    lo = b * BLK
                    hi = min(CH, lo + BLK)
                    wb = hi - lo
                    first = c == 0 and b == 0
                    last = c == NCH - 1 and b == nblk - 1
                    nc.tensor.matmul(
                        pb1[0:wb, 0:wb], v[:, lo:hi], rm[:, lo:hi],
                        start=first, stop=last,
                    )
                    nc.tensor.matmul(
                        pb2[0:wb, 0:wb], qm[:, lo:hi], qm[:, lo:hi],
                        start=first, stop=last,
                    )

            mm = cp.tile([ML, 2 * BLK], F32)
            nc.vector.tensor_copy(mm[:, 0:BLK], pb1[:])
            nc.vector.tensor_copy(mm[:, BLK : 2 * BLK], pb2[:])
            nc.sync.dma_start(mm_d.ap(), mm[:])
            nc.sync.dma_start(acc_d.ap(), acc[:])

    nc.compile()
    return nc


_NC_CACHE = None


def _get_program():
    global _NC_CACHE
    if _NC_CACHE is None:
        _NC_CACHE = _build_program()
    return _NC_CACHE


def _reference_numpy(out_moments, tgt_moments, num_targets, iou2ds, mask2d):
    """Exact numpy replica of the jax reference (fallback path)."""
    M_, N_, _ = iou2ds.shape
    S_, P_, _ = out_moments.shape
    scatter = np.repeat(np.arange(S_), num_targets)
    om = out_moments[scatter].astype(np.float32)      # [M, P, 2]
    tg = tgt_moments[:, None, :].astype(np.float32)
    s1, e1 = om[..., 0], om[..., 1]
    s2, e2 = tg[..., 0], tg[..., 1]
    inter = np.clip(np.minimum(e1, e2) - np.maximum(s1, s2), 0.0, None)
    union = (e1 - s1) + (e2 - s2) - inter
    iou = inter / union
    enclose = np.maximum(e1, e2) - np.minimum(s1, s2)
    cdist = (s1 + e1) * 0.5 - (s2 + e2) * 0.5
    bbox_diou = iou - (cdist * cdist) / (enclose * enclose)
    flat_idx = np.nonzero(mask2d.reshape(-1))[0]
    iou1 = iou2ds.reshape(M_, -1)[:, flat_idx]
    kth = np.argpartition(-iou1, TOPK - 1, axis=1)[:, :TOPK]
    target_mask = np.zeros((M_, P_), np.float32)
    target_mask[np.arange(M_)[:, None], kth] = 1.0
    target_mask = np.where(iou1 > IOU_THRESHOLD, 1.0, target_mask)
    loss = 1.0 - bbox_diou
    return np.float32((loss * target_mask).sum() / target_mask.sum())


def kernel(out_moments, tgt_moments, num_targets, iou2ds, mask2d):
    out_moments = np.asarray(out_moments, np.float32)
    tgt_moments = np.asarray(tgt_moments, np.float32)
    num_targets = np.asarray(num_targets, np.int32)
    iou2ds = np.asarray(iou2ds, np.float32)
    mask2d_np = np.asarray(mask2d)

    uniform = bool(np.all(num_targets == T))
    triu_ok = bool(
        np.array_equal(mask2d_np, np.triu(np.ones((N, N), dtype=bool)))
    )
    if not (uniform and triu_ok and iou2ds.shape == (M, N, N)):
        return _reference_numpy(
            out_moments, tgt_moments, num_targets, iou2ds, mask2d_np
        )

    nc = _get_program()
    bf16 = ml_dtypes.bfloat16

    # host layout prep: triu-compact iou2ds to p-order, bf16
    flat_idx = np.nonzero(mask2d_np.reshape(-1))[0]
    iou1 = iou2ds.reshape(M, -1)[:, flat_idx].astype(bf16)   # [M, P]
    s1 = out_moments[..., 0]                                  # [S, P] f32
    e1 = out_moments[..., 1]
    c1 = (s1 + e1).astype(bf16)
    s1 = s1.astype(bf16)
    e1 = e1.astype(bf16)

    in_maps = []
    for k in range(NCORES):
        sl_m = slice(k * ML, (k + 1) * ML)
        sl_s = slice(k * W, (k + 1) * W)
        # replicate each sample's moments across its 16 target partitions
        s1k = np.repeat(s1[sl_s], T, axis=0)                  # [128, P]
        e1k = np.repeat(e1[sl_s], T, axis=0)
        c1k = np.repeat(c1[sl_s], T, axis=0)
        iouk = iou1[sl_m]
        pio = np.empty((NCH, ML, CH), bf16)
        pmo = np.empty((NCH, ML, 3 * CH), bf16)
        for c in range(NCH):
            sl_p = slice(c * CH, (c + 1) * CH)
            pio[c] = iouk[:, sl_p]
            pmo[c, :, 0:CH] = s1k[:, sl_p]
            pmo[c, :, CH : 2 * CH] = e1k[:, sl_p]
            pmo[c, :, 2 * CH : 3 * CH] = c1k[:, sl_p]
        tgtk = tgt_moments[sl_m]                              # [128, 2] f32
        tgt3 = np.empty((ML, 3), np.float32)
        tgt3[:, 0] = tgtk[:, 0]
        tgt3[:, 1] = tgtk[:, 1]
        tgt3[:, 2] = tgtk[:, 0] + tgtk[:, 1]
        in_maps.append(
            {
                "iou": np.ascontiguousarray(pio.reshape(NCH * ML, CH)),
                "mo": np.ascontiguousarray(pmo.reshape(NCH * ML, 3 * CH)),
                "tgt": tgt3,
            }
        )

    trace = bool(int(os.environ.get("BBK_TRACE", "0")))
    res = bass_utils.run_bass_kernel_spmd(
        nc, in_maps, core_ids=list(range(NCORES)), trace=trace
    )
    if trace:
        kernel.last_exec_time_ns = res.exec_time_ns

    acc = np.stack([res.results[k]["acc"] for k in range(NCORES)])  # [8,128,NCH]
    mm = np.stack([res.results[k]["mm"] for k in range(NCORES)])    # [8,128,2*BLK]
    acc64 = acc.astype(np.float64)
    a_rows = acc64.sum(axis=2)                     # per-core per-row counts
    A = a_rows.sum()
    mm64 = mm.astype(np.float64)
    B1 = np.trace(mm64[:, :, 0:BLK], axis1=1, axis2=2).sum()
    B2 = np.trace(mm64[:, :, BLK : 2 * BLK], axis1=1, axis2=2).sum() / 4.0

    if a_rows.min() < TOPK:
        # top-3 not subsumed by the threshold for some row: replicate the
        # reference exactly on host (rare/degenerate inputs only).
        return _reference_numpy(
            out_moments, tgt_moments, num_targets, iou2ds, mask2d_np
        )

    return np.float32((A - B1 + B2) / A)


# revision 25
# speedup vs baseline: 1.0302x; 1.0302x over previous
"""Bass/Trainium2 kernel for nn_BboxIoULoss (topk_masking).

Computes, for S=64 samples / M=1024 targets / P=8256 triu proposals:
    loss = sum((1 - diou) * mask) / sum(mask)
where mask = topk-scatter(3) OR (iou1ds > 0.5), iou1ds = triu-gather of
iou2ds, and diou is the 1-D DIoU between each target and the per-sample
proposal moments.

Strategy (8 NeuronCores, data-parallel over M):
  - core k handles targets m in [128k, 128(k+1)); partition p = local
    target; each partition's sample is p // 16.
  - host prep (sharding/layout only): triu-compact iou2ds to p-order,
    cast to bf16; replicate per-sample moments s1/e1/c1=s1+e1 (bf16)
    across each sample's 16 target partitions; chunk-major slabs so
    each chunk is two contiguous DMAs (iou | moments).
  - device math per chunk (bf16; union == enclose whenever the boxes
    overlap, so iou = relu(w)/enc), tiles reused in place:
        mask = is_gt(iou, 0.5)                 DVE ts (4x)
        rowA += sum(mask)                      ACT copy (+accum, in place)
        u    = max(s1, s2)                     DVE ts
        v    = min(e1, e2)                     DVE ts
        v    = v - u          (w)              DVE tt (2x)
        mn   = min(s1, s2)                     DVE ts
        mx   = max(e1, e2)                     DVE ts
        mx   = mx - mn        (enc)            DVE tt
        lg   = Ln(mx)                          ACT
        renc = Exp(-lg)       (1/enc)          ACT (~2 ULP splines)
        mask = mask * renc    (rm)             DVE tt
        v    = max(v, 0)      (rw)             DVE ts
        cd   = c1 - c2                         DVE ts
        cd   = cd * rm        (qm)             DVE tt
    and the two big sums go to the otherwise-idle TensorEngine as
    block-diagonal PSUM accumulations over 128-column blocks:
        PB1 += rw_blk^T @ rm_blk      (diag holds sum(rw*rm) partials)
        PB2 += qm_blk^T @ qm_blk      (diag holds sum(qm^2) partials)
    B1 = trace(PB1) = sum(mask*iou), B2q = trace(PB2) = 4*sum(mask*pen),
        answer = (A - B1 + B2q/4) / A   (host sums in float64).
  - the top-3 scatter is subsumed by the threshold whenever every row
    has >= 3 entries above 0.5 (then the top-3 values are all > 0.5).
    The device returns per-row counts; if any row has < 3, or
    num_targets is not uniform, a numpy fallback reproduces the
    reference exactly.
"""

import os
import ml_dtypes
import numpy as np

import concourse.bass as bass
import concourse.tile as tile
import concourse.mybir as mybir
from concourse import bacc, bass_utils

F32 = mybir.dt.float32
BF16 = mybir.dt.bfloat16
AF = mybir.ActivationFunctionType
OP = mybir.AluOpType

S = 64
T = 16
N = 128
M = S * T                  # 1024
P = N * (N + 1) // 2       # 8256
TOPK = 3
IOU_THRESHOLD = 0.5
NCORES = 8
ML = M // NCORES           # 128 targets / core (= partitions)
W = S // NCORES            # 8 samples / core
NCH = int(os.environ.get("BBK_NCH", "4"))
CH = P // NCH              # free-dim chunk
BLK = 128                  # matmul block width


def _patch_act_tables():
    """Force one activation table-set (has ln/exp/relu/square/copy) so the
    scheduler emits a single ACT_TABLE_LOAD instead of thrashing sets."""
    import concourse.bacc as _bacc
    orig = _bacc.get_activation_tables

    def only_lnexp(arch):
        tabs = orig(arch)
        name = "natural_log_exp_and_others"
        if name not in tabs:
            return tabs
        return {k: (v if k == name else set()) for k, v in tabs.items()}

    _bacc.get_activation_tables = only_lnexp


def _build_program():
    if not os.environ.get("BBK_NOPATCH"):
        _patch_act_tables()
    nc = bacc.Bacc(
        "TRN2", target_bir_lowering=False, debug=False, enable_asserts=False
    )
    # chunk-major slabs: rows [c*ML, (c+1)*ML) = chunk c
    iou_d = nc.dram_tensor("iou", [NCH * ML, CH], BF16, kind="ExternalInput")
    mo_d = nc.dram_tensor("mo", [NCH * ML, 3 * CH], BF16, kind="ExternalInput")
    tgt_d = nc.dram_tensor("tgt", [ML, 3], F32, kind="ExternalInput")
    # acc: cols [0:NCH] = per-chunk rowA
    acc_d = nc.dram_tensor("acc", [ML, NCH], F32, kind="ExternalOutput")
    # [PB1 | PB2] block-diagonal gram accumulators (host takes traces)
    mm_d = nc.dram_tensor("mm", [ML, 2 * BLK], F32, kind="ExternalOutput")

    linearize = bool(int(os.environ.get("BBK_LINEARIZE", "0")))
    with tile.TileContext(nc, linearize=linearize) as tc:
        with (
            tc.tile_pool(name="const", bufs=1) as cp,
            tc.tile_pool(name="inp", bufs=int(os.environ.get("BBK_IBUFS", "3"))) as ip,
            tc.tile_pool(name="work", bufs=int(os.environ.get("BBK_WBUFS", "3"))) as wp,
            tc.psum_pool(name="ps", bufs=1) as pp,
        ):
            tgt = cp.tile([ML, 3], F32)
            s2 = tgt[:, 0:1]
            e2 = tgt[:, 1:2]
            c2 = tgt[:, 2:3]

            acc = cp.tile([ML, NCH], F32)
            pb1 = pp.tile([ML, BLK], F32)
            pb2 = pp.tile([ML, BLK], F32)

            cd_eng = os.environ.get("BBK_CD_ENGINE", "vector")
            nblk = (CH + BLK - 1) // BLK
            for c in range(NCH):
                iot = ip.tile([ML, CH], BF16, tag="iot")
                nc.sync.dma_start(
                    iot[:], iou_d.ap()[c * ML : (c + 1) * ML, :]
                )
                mot = ip.tile([ML, 3 * CH], BF16, tag="mot")
                nc.sync.dma_start(
                    mot[:], mo_d.ap()[c * ML : (c + 1) * ML, :]
                )
                if c == 0:
                    # after the chunk-0 slabs so their transfers start first
                    nc.sync.dma_start(tgt[:], tgt_d.ap())
                s1 = mot[:, 0:CH]
                e1 = mot[:, CH : 2 * CH]
                c1 = mot[:, 2 * CH : 3 * CH]

                # mask -> (in place) rowA accum -> (in place) rm
                mask = wp.tile([ML, CH], BF16, tag="mask")
                nc.vector.tensor_scalar(
                    mask[:], iot[:], IOU_THRESHOLD, None, OP.is_gt
                )
                nc.scalar.activation(
                    mask[:], mask[:], AF.Copy,
                    accum_out=acc[:, c : c + 1],
                )

                u = wp.tile([ML, CH], BF16, tag="u")
                nc.vector.tensor_scalar(u[:], s1, s2, None, OP.max)
                v = wp.tile([ML, CH], BF16, tag="v")
                nc.vector.tensor_scalar(v[:], e1, e2, None, OP.min)
                nc.vector.tensor_tensor(v[:], v[:], u[:], OP.subtract)  # w
                mn = wp.tile([ML, CH], BF16, tag="mn")
                nc.vector.tensor_scalar(mn[:], s1, s2, None, OP.min)
                mx = wp.tile([ML, CH], BF16, tag="mx")
                nc.vector.tensor_scalar(mx[:], e1, e2, None, OP.max)
                nc.vector.tensor_tensor(mx[:], mx[:], mn[:], OP.subtract)  # enc

                lg = wp.tile([ML, CH], F32, tag="lg")
                nc.scalar.activation(lg[:], mx[:], AF.Ln)
                renc = wp.tile([ML, CH], BF16, tag="renc")
                nc.scalar.activation(renc[:], lg[:], AF.Exp, scale=-1.0)

                if os.environ.get("BBK_RM_ENGINE", "vector") == "gpsimd":
                    nc.gpsimd.tensor_tensor(mask[:], mask[:], renc[:], OP.mult)
                else:
                    nc.vector.tensor_tensor(mask[:], mask[:], renc[:], OP.mult)
                rm = mask  # rm overwrote mask in place

                cd = wp.tile([ML, CH], BF16, tag="cd")
                if cd_eng == "gpsimd":
                    nc.gpsimd.tensor_scalar(cd[:], c1, c2, None, OP.subtract)
                else:
                    nc.vector.tensor_scalar(cd[:], c1, c2, None, OP.subtract)
                nc.vector.tensor_tensor(cd[:], cd[:], rm[:], OP.mult)  # qm
                qm = cd

                nc.vector.tensor_scalar(v[:], v[:], 0.0, None, OP.max)  # rw
                for b in range(nblk):
                    lo = b * BLK
                    hi = min(CH, lo + BLK)
                    wb = hi - lo
                    first = c == 0 and b == 0
                    last = c == NCH - 1 and b == nblk - 1
                    nc.tensor.matmul(
                        pb1[0:wb, 0:wb], v[:, lo:hi], rm[:, lo:hi],
                        start=first, stop=last,
                    )
                    nc.tensor.matmul(
                        pb2[0:wb, 0:wb], qm[:, lo:hi], qm[:, lo:hi],
                        start=first, stop=last,
                    )

            mm = cp.tile([ML, 2 * BLK], F32)
            nc.vector.tensor_copy(mm[:, 0:BLK], pb1[:])
            nc.vector.tensor_copy(mm[:, BLK : 2 * BLK], pb2[:])
            nc.sync.dma_start(mm_d.ap(), mm[:])
            nc.sync.dma_start(acc_d.ap(), acc[:])

    nc.compile()
    return nc


_NC_CACHE = None


def _get_program():
    global _NC_CACHE
    if _NC_CACHE is None:
        _NC_CACHE = _build_program()
    return _NC_CACHE


def _reference_numpy(out_moments, tgt_moments, num_targets, iou2ds, mask2d):
    """Exact numpy replica of the jax reference (fallback path)."""
    M_, N_, _ = iou2ds.shape
    S_, P_, _ = out_moments.shape
    scatter = np.repeat(np.arange(S_), num_targets)
    om = out_moments[scatter].astype(np.float32)      # [M, P, 2]
    tg = tgt_moments[:, None, :].astype(np.float32)
    s1, e1 = om[..., 0], om[..., 1]
    s2, e2 = tg[..., 0], tg[..., 1]
    inter = np.clip(np.minimum(e1, e2) - np.maximum(s1, s2), 0.0, None)
    union = (e1 - s1) + (e2 - s2) - inter
    iou = inter / union
    enclose = np.maximum(e1, e2) - np.minimum(s1, s2)
    cdist = (s1 + e1) * 0.5 - (s2 + e2) * 0.5
    bbox_diou = iou - (cdist * cdist) / (enclose * enclose)
    flat_idx = np.nonzero(mask2d.reshape(-1))[0]
    iou1 = iou2ds.reshape(M_, -1)[:, flat_idx]
    kth = np.argpartition(-iou1, TOPK - 1, axis=1)[:, :TOPK]
    target_mask = np.zeros((M_, P_), np.float32)
    target_mask[np.arange(M_)[:, None], kth] = 1.0
    target_mask = np.where(iou1 > IOU_THRESHOLD, 1.0, target_mask)
    loss = 1.0 - bbox_diou
    return np.float32((loss * target_mask).sum() / target_mask.sum())


def kernel(out_moments, tgt_moments, num_targets, iou2ds, mask2d):
    out_moments = np.asarray(out_moments, np.float32)
    tgt_moments = np.asarray(tgt_moments, np.float32)
    num_targets = np.asarray(num_targets, np.int32)
    iou2ds = np.asarray(iou2ds, np.float32)
    mask2d_np = np.asarray(mask2d)

    uniform = bool(np.all(num_targets == T))
    triu_ok = bool(
        np.array_equal(mask2d_np, np.triu(np.ones((N, N), dtype=bool)))
    )
    if not (uniform and triu_ok and iou2ds.shape == (M, N, N)):
        return _reference_numpy(
            out_moments, tgt_moments, num_targets, iou2ds, mask2d_np
        )

    nc = _get_program()
    bf16 = ml_dtypes.bfloat16

    # host layout prep: triu-compact iou2ds to p-order, bf16
    flat_idx = np.nonzero(mask2d_np.reshape(-1))[0]
    iou1 = iou2ds.reshape(M, -1)[:, flat_idx].astype(bf16)   # [M, P]
    s1 = out_moments[..., 0]                                  # [S, P] f32
    e1 = out_moments[..., 1]
    c1 = (s1 + e1).astype(bf16)
    s1 = s1.astype(bf16)
    e1 = e1.astype(bf16)

    in_maps = []
    for k in range(NCORES):
        sl_m = slice(k * ML, (k + 1) * ML)
        sl_s = slice(k * W, (k + 1) * W)
        # replicate each sample's moments across its 16 target partitions
        s1k = np.repeat(s1[sl_s], T, axis=0)                  # [128, P]
        e1k = np.repeat(e1[sl_s], T, axis=0)
        c1k = np.repeat(c1[sl_s], T, axis=0)
        iouk = iou1[sl_m]
        pio = np.empty((NCH, ML, CH), bf16)
        pmo = np.empty((NCH, ML, 3 * CH), bf16)
        for c in range(NCH):
            sl_p = slice(c * CH, (c + 1) * CH)
            pio[c] = iouk[:, sl_p]
            pmo[c, :, 0:CH] = s1k[:, sl_p]
            pmo[c, :, CH : 2 * CH] = e1k[:, sl_p]
            pmo[c, :, 2 * CH : 3 * CH] = c1k[:, sl_p]
        tgtk = tgt_moments[sl_m]                              # [128, 2] f32
        tgt3 = np.empty((ML, 3), np.float32)
        tgt3[:, 0] = tgtk[:, 0]
        tgt3[:, 1] = tgtk[:, 1]
        tgt3[:, 2] = tgtk[:, 0] + tgtk[:, 1]
        in_maps.append(
            {
                "iou": np.ascontiguousarray(pio.reshape(NCH * ML, CH)),
                "mo": np.ascontiguousarray(pmo.reshape(NCH * ML, 3 * CH)),
                "tgt": tgt3,
            }
        )

    trace = bool(int(os.environ.get("BBK_TRACE", "0")))
    res = bass_utils.run_bass_kernel_spmd(
        nc, in_maps, core_ids=list(range(NCORES)), trace=trace
    )
    if trace:
        kernel.last_exec_time_ns = res.exec_time_ns

    acc = np.stack([res.results[k]["acc"] for k in range(NCORES)])  # [8,128,NCH]
    mm = np.stack([res.results[k]["mm"] for k in range(NCORES)])    # [8,128,2*BLK]
    acc64 = acc.astype(np.float64)
    a_rows = acc64.sum(axis=2)                     # per-core per-row counts
    A = a_rows.sum()
    mm64 = mm.astype(np.float64)
    B1 = np.trace(mm64[:, :, 0:BLK], axis1=1, axis2=2).sum()
    B2 = np.trace(mm64[:, :, BLK : 2 * BLK], axis1=1, axis2=2).sum() / 4.0

    if a_rows.min() < TOPK:
        # top-3 not subsumed by the threshold for some row: replicate the
        # reference exactly on host (rare/degenerate inputs only).
        return _reference_numpy(
            out_moments, tgt_moments, num_targets, iou2ds, mask2d_np
        )

    return np.float32((A - B1 + B2) / A)
